# revision 1
# baseline (speedup 1.0000x reference)
"""MoE FFN (8 experts, top-2, SwiGLU) Trainium2 kernel.

Expert-parallel sharding: core e holds expert e's weights. Tokens are
dispatched (all-to-all style, decided on host from the router scores) to the
cores owning their top-2 experts; each core runs the router + SwiGLU FFN for
its gathered tokens on device and applies its own combine weight. The host
scatter-adds the per-expert partial outputs back into token order.

The router matmul runs in plain fp32 so the device's top-2 selection matches
the reference bit-closely (top-2 prob gaps can be ~1e-5); the heavyweight FFN
matmuls run in fp32r (full PE rate, ~12-bit mantissa, rel err ~1e-4). fp32r
operands are pre-rounded on the host (exact static_cast_fp32_to_fp32r) and
fed through float32r DRAM tensors, which satisfies the BIR verifier's
"rounded to FP32r" producer rule for DMA-loaded tiles.

Self-contained: shapes/sharding hardcoded for
x[2,2048,1024], 8 experts, d_expert=2048, top-2.
"""

import math
from contextlib import ExitStack

import ml_dtypes
import numpy as np

import concourse.bass as bass
import concourse.mybir as mybir
import concourse.tile as tile
from concourse import bacc
from concourse.bass_utils import run_bass_kernel_spmd
from concourse.masks import make_identity
from neuron_dtypes import static_cast_fp32_to_fp32r

# ---- problem constants --------------------------------------------------
B, T, D = 2, 2048, 1024
N_TOK = B * T          # 4096 tokens
E = 8                  # experts == cores
H = 2048               # expert hidden dim
TOP_K = 2
P = 128

CAP = 1152             # per-expert token capacity per dispatch round
ND = D // P            # 8  d-tiles (contraction tiles of d_model)
NH = H // P            # 16 h-tiles
NT = CAP // P          # 10 token tiles
NDC = D // 512         # 2  output column chunks

FP = mybir.dt.float32
FR = mybir.dt.float32r
BF = mybir.dt.bfloat16
AX = mybir.AxisListType.X
AF = mybir.ActivationFunctionType
OP = mybir.AluOpType

# moving-dim chunks of CAP; every chunk >= 256 keeps fp32r at full PE rate
_A_CHUNKS = [(0, 512), (512, 384), (896, 256)]
assert sum(w for _, w in _A_CHUNKS) == CAP


def _round_fp32r(a):
    """Exact host-side fp32 -> fp32r rounding (bit layout stays fp32)."""
    return static_cast_fp32_to_fp32r(np.ascontiguousarray(a, dtype=np.float32)).view(
        np.float32
    )


def _emit(nc, tc, ctx, xtf_d, xtr_d, wg_d, wv_d, wo_d, gw_d, ge_d, gwb_d,
          eb_d, esel_d, y_d):
    const = ctx.enter_context(tc.tile_pool(name="const", bufs=1))
    xf_pool = ctx.enter_context(tc.tile_pool(name="xf", bufs=4))
    xtr_pool = ctx.enter_context(tc.tile_pool(name="xtr", bufs=1))
    ht_pool = ctx.enter_context(tc.tile_pool(name="ht", bufs=1))
    wgv_pool = ctx.enter_context(tc.tile_pool(name="wgv", bufs=3))
    wo_pool = ctx.enter_context(tc.tile_pool(name="wo", bufs=1))
    act_pool = ctx.enter_context(tc.tile_pool(name="act", bufs=3))
    yst_pool = ctx.enter_context(tc.tile_pool(name="yst", bufs=3))
    rt = ctx.enter_context(tc.tile_pool(name="rt", bufs=2))

    # ---- small resident constants --------------------------------------
    gw_sb = const.tile([P, ND, E], FR)
    nc.sync.dma_start(out=gw_sb[:], in_=gw_d.ap().rearrange("p (dt e) -> p dt e", dt=ND))
    ge_sb = const.tile([P, ND, E], FR)
    nc.sync.dma_start(out=ge_sb[:], in_=ge_d.ap().rearrange("p (dt e) -> p dt e", dt=ND))
    gwb_sb = const.tile([P, ND, E], BF)
    nc.sync.dma_start(out=gwb_sb[:], in_=gwb_d.ap().rearrange("p (dt e) -> p dt e", dt=ND))
    ident = const.tile([P, P], FP)
    make_identity(nc, ident[:])

    # combine weight per token tile, written column by column
    w_sb = const.tile([P, NT], FP)
    # transposed logits staging [E, CAP]
    ltT = const.tile([E, CAP], FP)

    # x loads, interleaved: router fp32 stream (xf) + FFN fp32r (xtr)
    xf_sb = []
    xtr_sb = []
    for d in range(ND):
        xf_t = xf_pool.tile([P, CAP], BF, tag="xf", name=f"xe{d}")
        nc.sync.dma_start(out=xf_t[:], in_=xtf_d.ap()[d * P:(d + 1) * P, :])
        xf_sb.append(xf_t)
        xr = xtr_pool.tile([P, CAP], FR, tag=f"xr{d}")
        nc.sync.dma_start(out=xr[:], in_=xtr_d.ap()[d * P:(d + 1) * P, :])
        xtr_sb.append(xr)
    eb_sb = const.tile([P, E], FP)
    nc.scalar.dma_start(out=eb_sb[:], in_=eb_d.ap().partition_broadcast(P))
    esel_sb = const.tile([P, E], FP)
    nc.scalar.dma_start(out=esel_sb[:], in_=esel_d.ap().partition_broadcast(P))

    # ---- router: fp32 logits (transposed), then per-token-tile top-2 ----
    with ExitStack() as rctx:
        ps_l = rctx.enter_context(tc.tile_pool(name="psl", bufs=1, space="PSUM"))
        ps_t = rctx.enter_context(tc.tile_pool(name="pst", bufs=2, space="PSUM"))

        # PE warm-up: ~6us of junk matmuls on the resident identity tile
        # ramps the HAM clock gate before the serial fp32 router matmuls.
        warm = ps_t.tile([E, P], FP, name="warm", tag="warm")
        for _ in range(32):
            nc.tensor.matmul(
                warm[:], lhsT=ident[:, :E], rhs=ident[:],
                start=True, stop=True,
            )
        pslT = [
            ps_l.tile([E, 512], FP, name=f"pslT{ci}", tag=f"psl{ci}")
            for ci in range(len(_A_CHUNKS))
        ]
        for d in range(ND):
            for ci, (cs, cw) in enumerate(_A_CHUNKS):
                csl = slice(cs, cs + cw)
                nc.tensor.matmul(
                    pslT[ci][:, :cw], lhsT=gw_sb[:, d, :],
                    rhs=xtr_sb[d][:, csl], start=(d == 0), stop=False,
                )
                nc.tensor.matmul(
                    pslT[ci][:, :cw], lhsT=ge_sb[:, d, :],
                    rhs=xtr_sb[d][:, csl], start=False, stop=False,
                )
                nc.tensor.matmul(
                    pslT[ci][:, :cw], lhsT=gwb_sb[:, d, :],
                    rhs=xf_sb[d][:, csl], start=False, stop=(d == ND - 1),
                )
        for ci, (cs, cw) in enumerate(_A_CHUNKS):
            nc.vector.tensor_copy(ltT[:, cs:cs + cw], pslT[ci][:, :cw])

        for tt in range(NT):
            pst = ps_t.tile([P, E], FP)
            nc.tensor.transpose(
                pst[:], ltT[:, tt * P:(tt + 1) * P], ident[:E, :E]
            )
            logit = rt.tile([P, E], FP, tag="logit")
            nc.vector.tensor_add(logit[:], pst[:], eb_sb[:])
            mx1 = rt.tile([P, 1], FP, tag="mx1")
            nc.vector.reduce_max(mx1[:], logit[:], axis=AX)
            m1 = rt.tile([P, E], FP, tag="m1")
            nc.vector.tensor_scalar(m1[:], logit[:], mx1[:], None, op0=OP.is_equal)
            # knock out the argmax column, then take the second max
            big = rt.tile([P, E], FP, tag="big")
            nc.vector.tensor_scalar_mul(big[:], m1[:], 1e30)
            masked = rt.tile([P, E], FP, tag="masked")
            nc.vector.tensor_sub(masked[:], logit[:], big[:])
            mx2 = rt.tile([P, 1], FP, tag="mx2")
            nc.vector.reduce_max(mx2[:], masked[:], axis=AX)
            m2 = rt.tile([P, E], FP, tag="m2")
            nc.vector.tensor_scalar(m2[:], logit[:], mx2[:], None, op0=OP.is_equal)
            # softmax pieces: p1 = 1/Z, p2 = exp(mx2-mx1)/Z, Z = sum exp(l-mx1)
            nm1 = rt.tile([P, 1], FP, tag="nm1")
            nc.vector.tensor_scalar_mul(nm1[:], mx1[:], -1.0)
            zf = rt.tile([P, E], FP, tag="zf")
            nc.scalar.activation(zf[:], logit[:], AF.Exp, bias=nm1[:])
            zs = rt.tile([P, 1], FP, tag="zs")
            nc.vector.reduce_sum(zs[:], zf[:], axis=AX)
            e2 = rt.tile([P, 1], FP, tag="e2")
            nc.scalar.activation(e2[:], mx2[:], AF.Exp, bias=nm1[:])
            p1 = rt.tile([P, 1], FP, tag="p1")
            nc.vector.reciprocal(p1[:], zs[:])
            p2 = rt.tile([P, 1], FP, tag="p2")
            nc.vector.tensor_mul(p2[:], e2[:], p1[:])
            den = rt.tile([P, 1], FP, tag="den")
            nc.vector.tensor_add(den[:], p1[:], p2[:])
            nc.vector.tensor_scalar_add(den[:], den[:], 1e-8)
            rden = rt.tile([P, 1], FP, tag="rden")
            nc.vector.reciprocal(rden[:], den[:])
            w1 = rt.tile([P, 1], FP, tag="w1")
            nc.vector.tensor_mul(w1[:], p1[:], rden[:])
            w2 = rt.tile([P, 1], FP, tag="w2")
            nc.vector.tensor_mul(w2[:], p2[:], rden[:])
            # pick this core's expert via the one-hot selector
            s1 = rt.tile([P, E], FP, tag="s1")
            nc.vector.tensor_mul(s1[:], m1[:], esel_sb[:])
            is1 = rt.tile([P, 1], FP, tag="is1")
            nc.vector.reduce_sum(is1[:], s1[:], axis=AX)
            s2 = rt.tile([P, E], FP, tag="s2")
            nc.vector.tensor_mul(s2[:], m2[:], esel_sb[:])
            is2 = rt.tile([P, 1], FP, tag="is2")
            nc.vector.reduce_sum(is2[:], s2[:], axis=AX)
            wa = rt.tile([P, 1], FP, tag="wa")
            nc.vector.tensor_mul(wa[:], w1[:], is1[:])
            wb = rt.tile([P, 1], FP, tag="wb")
            nc.vector.tensor_mul(wb[:], w2[:], is2[:])
            nc.vector.tensor_tensor(w_sb[:, tt:tt + 1], wa[:], wb[:], op=OP.add)

    with ExitStack() as fctx:
        ps_g = fctx.enter_context(tc.tile_pool(name="psg", bufs=2, space="PSUM"))
        ps_v = fctx.enter_context(tc.tile_pool(name="psv", bufs=2, space="PSUM"))
        ps_y = fctx.enter_context(tc.tile_pool(name="psy", bufs=2, space="PSUM"))

        # ---- phase A: hT[h, tok] = silu(x@wg)^T * (x@wv)^T --------------
        wg_ap = wg_d.ap().rearrange("(dt p) h -> p dt h", p=P)
        wv_ap = wv_d.ap().rearrange("(dt p) h -> p dt h", p=P)
        ht_sb = []
        for hk in range(NH):
            hs = slice(hk * P, (hk + 1) * P)
            wgt = wgv_pool.tile([P, ND, P], FR, tag="wg")
            nc.sync.dma_start(out=wgt[:], in_=wg_ap[:, :, hs])
            wvt = wgv_pool.tile([P, ND, P], FR, tag="wv")
            nc.sync.dma_start(out=wvt[:], in_=wv_ap[:, :, hs])
            ht = ht_pool.tile([P, CAP], FR, tag=f"h{hk}")
            ht_sb.append(ht)
            for (cs, cw) in _A_CHUNKS:
                cslice = slice(cs, cs + cw)
                pg = ps_g.tile([P, 512], FP)
                pv = ps_v.tile([P, 512], FP)
                for d in range(ND):
                    nc.tensor.matmul(
                        pg[:, :cw],
                        lhsT=wgt[:, d, :],
                        rhs=xtr_sb[d][:, cslice],
                        start=(d == 0),
                        stop=(d == ND - 1),
                    )
                for d in range(ND):
                    nc.tensor.matmul(
                        pv[:, :cw],
                        lhsT=wvt[:, d, :],
                        rhs=xtr_sb[d][:, cslice],
                        start=(d == 0),
                        stop=(d == ND - 1),
                    )
                # silu(g)*v = g*sigmoid(g)*v, decomposed (sim lacks Silu)
                sg = act_pool.tile([P, 512], FP, tag="sg")
                nc.scalar.activation(sg[:, :cw], pg[:, :cw], AF.Sigmoid)
                gs = act_pool.tile([P, 512], FP, tag="gs")
                nc.vector.tensor_tensor(gs[:, :cw], pg[:, :cw], sg[:, :cw], op=OP.mult)
                # fp32r rounding producer for the down-projection matmul
                ht_inst = nc.vector.tensor_tensor(
                    ht[:, cslice], pv[:, :cw], gs[:, :cw], op=OP.mult
                )
                if hk == 2 and cs == _A_CHUNKS[-1][0]:
                    wo_gate_inst = ht_inst

        # ---- phase B: y[tok, d] = (hT^T @ wo) * w -----------------------
        from concourse.bass import _add_dep_helper
        for dc in range(NDC):
            dslice = slice(dc * 512, (dc + 1) * 512)
            wor = []
            for hk in range(NH):
                wt = wo_pool.tile([P, 512], FR, tag=f"wo{hk}")
                dma_inst = (nc.gpsimd if dc == 0 else nc.sync).dma_start(
                    out=wt[:], in_=wo_d.ap()[hk * P:(hk + 1) * P, dslice]
                )
                if dc == 0:
                    # keep these off the HBM port while the head loads stream
                    _add_dep_helper(
                        dma_inst.ins, wo_gate_inst.ins, sync=True,
                        reason="delay wo loads past A warmup",
                    )
                wor.append(wt)
            for tt in range(NT):
                ts = slice(tt * P, (tt + 1) * P)
                py = ps_y.tile([P, 512], FP)
                for hk in range(NH):
                    nc.tensor.matmul(
                        py[:],
                        lhsT=ht_sb[hk][:, ts],
                        rhs=wor[hk][:],
                        start=(hk == 0),
                        stop=(hk == NH - 1),
                    )
                ysb = yst_pool.tile([P, 512], FP, tag="y")
                nc.scalar.activation(ysb[:], py[:], AF.Copy, scale=w_sb[:, tt:tt + 1])
                nc.sync.dma_start(out=y_d.ap()[ts, dslice], in_=ysb[:])


def _build():
    nc = bacc.Bacc("TRN2", target_bir_lowering=False, debug=False)
    xtf_d = nc.dram_tensor("xe", [D, CAP], BF, kind="ExternalInput")
    xtr_d = nc.dram_tensor("xtr", [D, CAP], FR, kind="ExternalInput")
    wg_d = nc.dram_tensor("wg", [D, H], FR, kind="ExternalInput")
    wv_d = nc.dram_tensor("wv", [D, H], FR, kind="ExternalInput")
    wo_d = nc.dram_tensor("wo", [H, D], FR, kind="ExternalInput")
    gw_d = nc.dram_tensor("gw", [P, ND * E], FR, kind="ExternalInput")
    ge_d = nc.dram_tensor("ge", [P, ND * E], FR, kind="ExternalInput")
    gwb_d = nc.dram_tensor("gwb", [P, ND * E], BF, kind="ExternalInput")
    eb_d = nc.dram_tensor("eb", [1, E], FP, kind="ExternalInput")
    esel_d = nc.dram_tensor("esel", [1, E], FP, kind="ExternalInput")
    y_d = nc.dram_tensor("y", [CAP, D], FP, kind="ExternalOutput")
    with tile.TileContext(nc) as tc:
        with ExitStack() as ctx:
            _emit(nc, tc, ctx, xtf_d, xtr_d, wg_d, wv_d, wo_d, gw_d, ge_d, gwb_d,
                  eb_d, esel_d, y_d)
    nc.compile()
    return nc


_NC = None


def _get_nc():
    global _NC
    if _NC is None:
        _NC = _build()
    return _NC


def _route(xf, gate_w, expert_bias):
    """Host-side replica of the reference router, for dispatch decisions."""
    logits = xf @ gate_w + expert_bias          # [N, E] fp32
    m = logits.max(axis=-1, keepdims=True)
    p = np.exp(logits - m)
    p /= p.sum(axis=-1, keepdims=True)
    # ties -> lower index first, matching jax.lax.top_k
    order = np.argsort(-p, axis=-1, kind="stable")[:, :TOP_K]
    return order


def kernel(x, gate_w, expert_bias, w_gate, w_value, w_out, _trace=False):
    x = np.asarray(x, dtype=np.float32)
    gate_w = np.asarray(gate_w, dtype=np.float32)
    expert_bias = np.asarray(expert_bias, dtype=np.float32)
    w_gate = np.asarray(w_gate, dtype=np.float32)
    w_value = np.asarray(w_value, dtype=np.float32)
    w_out = np.asarray(w_out, dtype=np.float32)

    xf = np.ascontiguousarray(x.reshape(N_TOK, D))
    order = _route(xf, gate_w, expert_bias)
    idx = [np.flatnonzero((order == e).any(axis=-1)) for e in range(E)]
    n_rounds = max(1, math.ceil(max(len(i) for i in idx) / CAP))

    nc = _get_nc()
    gw_tiled = np.ascontiguousarray(
        gate_w.reshape(ND, P, E).transpose(1, 0, 2).reshape(P, ND * E)
    )
    gw_r = _round_fp32r(gw_tiled)
    ge_r = _round_fp32r(gw_tiled - gw_r)
    gw_b = gw_tiled.astype(ml_dtypes.bfloat16)
    eb2 = expert_bias.reshape(1, E)
    wg_r = [_round_fp32r(w_gate[e]) for e in range(E)]
    wv_r = [_round_fp32r(w_value[e]) for e in range(E)]
    wo_r = [_round_fp32r(w_out[e]) for e in range(E)]
    out = np.zeros((N_TOK, D), dtype=np.float32)
    last = None
    for r in range(n_rounds):
        in_maps = []
        for e in range(E):
            ids = idx[e][r * CAP:(r + 1) * CAP]
            ids_p = np.zeros(CAP, dtype=np.int64)
            ids_p[: len(ids)] = ids
            xt = np.ascontiguousarray(xf[ids_p].T)
            xt_r = _round_fp32r(xt)
            xe_b = (xt - xt_r).astype(ml_dtypes.bfloat16)
            esel = np.zeros((1, E), dtype=np.float32)
            esel[0, e] = 1.0
            in_maps.append({
                "xe": xe_b,
                "xtr": xt_r,
                "wg": wg_r[e],
                "wv": wv_r[e],
                "wo": wo_r[e],
                "gw": gw_r,
                "ge": ge_r,
                "gwb": gw_b,
                "eb": eb2,
                "esel": esel,
            })
        res = run_bass_kernel_spmd(
            nc, in_maps, core_ids=list(range(E)),
            trace=bool(_trace), trace_cores=list(range(E)) if _trace else None,
        )
        last = res
        for e in range(E):
            ids = idx[e][r * CAP:(r + 1) * CAP]
            if len(ids):
                out[ids] += res.results[e]["y"][: len(ids)]
    if _trace:
        kernel.last_results = last
    return out.reshape(B, T, D)



# revision 4
# speedup vs baseline: 1.2371x; 1.2371x over previous
"""MoE FFN (8 experts, top-2, SwiGLU) Trainium2 kernel.

Expert-parallel sharding: core e holds expert e's weights. The router runs on
the host (it already must, to decide dispatch): top-2 selection + softmax
combine weights are computed in numpy fp32, and each core receives its
gathered tokens plus a per-token combine weight. The device does only the
dense SwiGLU FFN:

    phase A:  hT[h, tok] = silu(wg.T @ x) * (wv.T @ x)      (fp32r matmuls)
    phase B:  y[tok, d]  = (hT.T @ wo) * w[tok]             (bf16 matmuls)

fp32r operands are pre-rounded on the host (exact static_cast_fp32_to_fp32r)
and fed through float32r DRAM tensors. Phase B runs in bf16 (hidden + wo),
which is rate-neutral on the PE but halves wo DMA and SBUF; the extra
~0.3% rms error is far inside the 2e-2 gate.

Self-contained: shapes/sharding hardcoded for
x[2,2048,1024], 8 experts, d_expert=2048, top-2.
"""

import math
from contextlib import ExitStack

import ml_dtypes
import numpy as np

import concourse.bass as bass
import concourse.mybir as mybir
import concourse.tile as tile
from concourse import bacc
from concourse.bass_utils import run_bass_kernel_spmd
from concourse.masks import make_identity
from neuron_dtypes import static_cast_fp32_to_fp32r

# ---- problem constants --------------------------------------------------
B, T, D = 2, 2048, 1024
N_TOK = B * T          # 4096 tokens
E = 8                  # experts == cores
H = 2048               # expert hidden dim
TOP_K = 2
P = 128

CAP = 1092             # per-expert token capacity (>= max load 1091, even chunks)
ND = D // P            # 8  d-tiles (contraction tiles of d_model)
NH = H // P            # 16 h-tiles
NT = math.ceil(CAP / P)  # 9 token tiles (8 full + 67)
NDC = D // 512         # 2  output column chunks

FP = mybir.dt.float32
FR = mybir.dt.float32r
BF = mybir.dt.bfloat16
AF = mybir.ActivationFunctionType
OP = mybir.AluOpType

# token chunks of CAP for phase A; >= 256 keeps fp32r at full PE rate and
# even widths satisfy the fp32r ISA restriction (moving/dst n_step even)
_A_CHUNKS = [(0, 512), (512, 324), (836, 256)]
assert sum(w for _, w in _A_CHUNKS) == CAP


def _round_fp32r(a):
    """Exact host-side fp32 -> fp32r rounding (bit layout stays fp32)."""
    return static_cast_fp32_to_fp32r(np.ascontiguousarray(a, dtype=np.float32)).view(
        np.float32
    )


def _emit(nc, tc, ctx, xtr_d, wgv_d, wo_d, wc_d, y_d):
    const = ctx.enter_context(tc.tile_pool(name="const", bufs=1))
    xc_pool = ctx.enter_context(tc.tile_pool(name="xc", bufs=1))
    ht_pool = ctx.enter_context(tc.tile_pool(name="ht", bufs=1))
    wgv_pool = ctx.enter_context(tc.tile_pool(name="wgv", bufs=3))
    wo_pool = ctx.enter_context(tc.tile_pool(name="wo", bufs=2))
    act_pool = ctx.enter_context(tc.tile_pool(name="act", bufs=3))
    yst_pool = ctx.enter_context(tc.tile_pool(name="yst", bufs=3))

    # ---- tiny resident constants ---------------------------------------
    wc_sb = const.tile([P, NT], FP)
    nc.scalar.dma_start(out=wc_sb[:], in_=wc_d.ap())
    ident = const.tile([P, P], FP)
    make_identity(nc, ident[:])

    # ---- input streams (one big DMA each, sync HWDGE ring, FIFO) -------
    xtr_ap = xtr_d.ap().rearrange("p (dt c) -> p dt c", dt=ND)
    wgv_ap = wgv_d.ap().rearrange("p (hk gdt q) -> p hk gdt q", hk=NH, q=P)
    wo_ap = wo_d.ap().rearrange("p (hk dc j) -> p hk dc j", hk=NH, dc=NDC)

    wgv_sb = []
    xc_sb = []

    def load_wgv(hk):
        t = wgv_pool.tile([P, 2 * ND, P], FR, tag="wgv")
        nc.sync.dma_start(out=t[:], in_=wgv_ap[:, hk])
        wgv_sb.append(t)

    load_wgv(0)
    for ci, (cs, cw) in enumerate(_A_CHUNKS):
        t = xc_pool.tile([P, ND, cw], FR, tag=f"xc{ci}")
        nc.sync.dma_start(out=t[:], in_=xtr_ap[:, :, cs:cs + cw])
        xc_sb.append(t)
    for hk in range(1, NH):
        load_wgv(hk)
    wo_sb = []
    for dc in range(NDC):
        t = wo_pool.tile([P, NH, 512], BF, tag="wo")
        nc.sync.dma_start(out=t[:], in_=wo_ap[:, :, dc])
        wo_sb.append(t)

    with ExitStack() as fctx:
        ps_w = fctx.enter_context(tc.tile_pool(name="psw", bufs=1, space="PSUM"))
        ps_g = fctx.enter_context(tc.tile_pool(name="psg", bufs=2, space="PSUM"))
        ps_v = fctx.enter_context(tc.tile_pool(name="psv", bufs=2, space="PSUM"))
        ps_y = fctx.enter_context(tc.tile_pool(name="psy", bufs=2, space="PSUM"))

        # PE warm-up: junk matmuls on the resident identity tile ramp the
        # HAM clock gate while the first x/weight DMAs are in flight.
        warm = ps_w.tile([E, P], FP, name="warm", tag="warm")
        for _ in range(32):
            nc.tensor.matmul(
                warm[:], lhsT=ident[:, :E], rhs=ident[:],
                start=True, stop=True,
            )

        # ---- phase A: hT[h, tok] = silu(x@wg)^T * (x@wv)^T --------------
        ht_sb = []
        for hk in range(NH):
            wgvt = wgv_sb[hk]
            ht = ht_pool.tile([P, CAP], BF, tag=f"h{hk}")
            ht_sb.append(ht)
            for ci, (cs, cw) in enumerate(_A_CHUNKS):
                cslice = slice(cs, cs + cw)
                pg = ps_g.tile([P, 512], FP)
                pv = ps_v.tile([P, 512], FP)
                for dn in range(ND):
                    nc.tensor.matmul(
                        pg[:, :cw],
                        lhsT=wgvt[:, dn, :],
                        rhs=xc_sb[ci][:, dn, :],
                        start=(dn == 0),
                        stop=(dn == ND - 1),
                    )
                for dn in range(ND):
                    nc.tensor.matmul(
                        pv[:, :cw],
                        lhsT=wgvt[:, ND + dn, :],
                        rhs=xc_sb[ci][:, dn, :],
                        start=(dn == 0),
                        stop=(dn == ND - 1),
                    )
                # silu(g)*v = g*sigmoid(g)*v, decomposed (sim lacks Silu)
                sg = act_pool.tile([P, 512], FP, tag="sg")
                nc.scalar.activation(sg[:, :cw], pg[:, :cw], AF.Sigmoid)
                gs = act_pool.tile([P, 512], FP, tag="gs")
                nc.vector.tensor_tensor(gs[:, :cw], pg[:, :cw], sg[:, :cw], op=OP.mult)
                nc.vector.tensor_tensor(
                    ht[:, cslice], pv[:, :cw], gs[:, :cw], op=OP.mult
                )

        # ---- phase B: y[tok, d] = (hT^T @ wo) * w -----------------------
        for dc in range(NDC):
            dslice = slice(dc * 512, (dc + 1) * 512)
            for tt in range(NT):
                pt = min(P, CAP - tt * P)
                ts = slice(tt * P, tt * P + pt)
                py = ps_y.tile([P, 512], FP)
                for hk in range(NH):
                    nc.tensor.matmul(
                        py[:pt, :],
                        lhsT=ht_sb[hk][:, ts],
                        rhs=wo_sb[dc][:, hk, :],
                        start=(hk == 0),
                        stop=(hk == NH - 1),
                    )
                ysb = yst_pool.tile([P, 512], FP, tag="y")
                nc.vector.tensor_scalar(
                    ysb[:pt, :], py[:pt, :], wc_sb[:pt, tt:tt + 1], None,
                    op0=OP.mult,
                )
                nc.scalar.dma_start(out=y_d.ap()[ts, dslice], in_=ysb[:pt, :])


def _build():
    nc = bacc.Bacc("TRN2", target_bir_lowering=False, debug=False)
    xtr_d = nc.dram_tensor("xtr", [P, ND * CAP], FR, kind="ExternalInput")
    wgv_d = nc.dram_tensor("wgv", [P, NH * 2 * ND * P], FR, kind="ExternalInput")
    wo_d = nc.dram_tensor("wo", [P, NH * NDC * 512], BF, kind="ExternalInput")
    wc_d = nc.dram_tensor("wc", [P, NT], FP, kind="ExternalInput")
    y_d = nc.dram_tensor("y", [CAP, D], FP, kind="ExternalOutput")
    with tile.TileContext(nc) as tc:
        with ExitStack() as ctx:
            _emit(nc, tc, ctx, xtr_d, wgv_d, wo_d, wc_d, y_d)
    nc.compile()
    return nc


_NC = None


def _get_nc():
    global _NC
    if _NC is None:
        _NC = _build()
    return _NC


def _route(xf, gate_w, expert_bias):
    """Host-side replica of the reference router."""
    logits = xf @ gate_w + expert_bias          # [N, E] fp32
    m = logits.max(axis=-1, keepdims=True)
    p = np.exp(logits - m)
    p /= p.sum(axis=-1, keepdims=True)
    # ties -> lower index first, matching jax.lax.top_k
    order = np.argsort(-p, axis=-1, kind="stable")[:, :TOP_K]
    rw = np.take_along_axis(p, order, axis=-1)  # [N, K]
    rw = rw / (rw.sum(axis=-1, keepdims=True) + 1e-8)
    return order, rw


def _pack_wgv(wg_r, wv_r):
    """[D,H]x2 fp32r -> [P, NH*2*ND*P] matching the SBUF lhsT tile layout."""
    # target[p, hk, g, dn, q] = w_g[dn*P + p, hk*P + q]
    def tile4(w):
        return w.reshape(ND, P, NH, P).transpose(1, 2, 0, 3)  # [P, NH, ND, P]
    packed = np.stack([tile4(wg_r), tile4(wv_r)], axis=2)     # [P, NH, 2, ND, P]
    return np.ascontiguousarray(packed).reshape(P, -1)


def kernel(x, gate_w, expert_bias, w_gate, w_value, w_out, _trace=False):
    x = np.asarray(x, dtype=np.float32)
    gate_w = np.asarray(gate_w, dtype=np.float32)
    expert_bias = np.asarray(expert_bias, dtype=np.float32)
    w_gate = np.asarray(w_gate, dtype=np.float32)
    w_value = np.asarray(w_value, dtype=np.float32)
    w_out = np.asarray(w_out, dtype=np.float32)

    xf = np.ascontiguousarray(x.reshape(N_TOK, D))
    order, rw = _route(xf, gate_w, expert_bias)
    idx = [np.flatnonzero((order == e).any(axis=-1)) for e in range(E)]
    wtok = []
    for e in range(E):
        sel = (order[idx[e]] == e)
        wtok.append((rw[idx[e]] * sel).sum(axis=-1).astype(np.float32))
    n_rounds = max(1, math.ceil(max(len(i) for i in idx) / CAP))

    nc = _get_nc()
    wgv_e = [
        _pack_wgv(_round_fp32r(w_gate[e]), _round_fp32r(w_value[e]))
        for e in range(E)
    ]
    # wo[p, hk, dc, j] = w_out[e][hk*P + p, dc*512 + j], bf16
    wo_e = [
        np.ascontiguousarray(
            w_out[e].astype(ml_dtypes.bfloat16).reshape(NH, P, NDC, 512)
            .transpose(1, 0, 2, 3)
        ).reshape(P, -1)
        for e in range(E)
    ]
    out = np.zeros((N_TOK, D), dtype=np.float32)
    last = None
    for r in range(n_rounds):
        in_maps = []
        for e in range(E):
            ids = idx[e][r * CAP:(r + 1) * CAP]
            ids_p = np.zeros(CAP, dtype=np.int64)
            ids_p[: len(ids)] = ids
            xt = np.ascontiguousarray(xf[ids_p].T)           # [D, CAP]
            xt_r = _round_fp32r(xt)
            xtr = np.ascontiguousarray(
                xt_r.reshape(ND, P, CAP).transpose(1, 0, 2)
            ).reshape(P, -1)
            w_pad = np.zeros(NT * P, dtype=np.float32)
            w_pad[: len(ids)] = wtok[e][r * CAP:(r + 1) * CAP]
            wc = np.ascontiguousarray(w_pad.reshape(NT, P).T)  # [P, NT]
            in_maps.append({
                "xtr": xtr,
                "wgv": wgv_e[e],
                "wo": wo_e[e],
                "wc": wc,
            })
        res = run_bass_kernel_spmd(
            nc, in_maps, core_ids=list(range(E)),
            trace=bool(_trace), trace_cores=list(range(E)) if _trace else None,
        )
        last = res
        for e in range(E):
            ids = idx[e][r * CAP:(r + 1) * CAP]
            if len(ids):
                out[ids] += res.results[e]["y"][: len(ids)]
    if _trace:
        kernel.last_results = last
    return out.reshape(B, T, D)
